# revision 13
# baseline (speedup 1.0000x reference)
"""Trainium2 kernel: 100x100 sliding-window mean over [32,1,1124,1124] -> [32,1,1025,1025].

Strategy (per core, 4 images, pure data parallel over batch):
  1. Vertical 100-row box sum via TensorE band-matrix matmuls (fp32r, band
     stationary with scale 1e-4 folded in), accumulating 2 h-chunks in PSUM.
  2. Horizontal 100-col box sum via a single DVE tensor_tensor_scan per row
     tile: state = (t[w] + state) - t[w-100], using a 100-col zero prefix.
  3. Contiguous DMA in/out; ScalarE evacuates PSUM->SBUF.

Built on bacc.Bacc (its generate_event_semaphores pass splits multi-waits,
which raw Bass+Tile does not). Host pads x by 28 rows so each image loads as
9x128 rows in one DMA.
"""

import numpy as np

import concourse.bass as bass
from concourse import bacc
import concourse.mybir as mybir
import concourse.tile as tile
from concourse.bass_utils import run_bass_kernel_spmd

B = 32          # batch
H = W = 1124    # input spatial
K = 100         # window
OH = OW = H - K + 1  # 1025
PER = 4         # images per core
NCORES = 8
SCALE = np.float32(1.0 / (K * K))  # 1e-4
HPAD = 9 * 128 * PER - H * PER  # 28 pad rows so every image spans 9 full chunks
XROWS = H * PER + HPAD

F32 = mybir.dt.float32
F32R = mybir.dt.float32r

# w-chunks of the moving operand: (start, n). 1024..1280 streams 256 cols so
# fp32r runs at 1 cyc/row; cols >= 1124 are uninitialized garbage whose psum
# output columns are never read.
WCHUNKS = [(0, 512), (512, 512), (1024, 256)]
XW = 1280  # padded sbuf width per h-chunk


def _bands():
    """Band matrices (lhsT layout [h_rel, out_part]) for the vertical box sum.

    band_a: contribution of x chunk i to ho-tile i   (h = ho0 + hr)
    band_b: contribution of x chunk i+1 to ho-tile i (h = ho0 + 128 + hr)
    t[ho0+p, w] = SCALE * sum_{h=ho0+p}^{ho0+p+99} x[h, w]
    """
    hr = np.arange(128)[:, None]
    pr = np.arange(128)[None, :]
    a = ((pr <= hr) & (hr <= pr + 99)).astype(np.float32) * SCALE
    b = ((pr <= hr + 128) & (hr + 128 <= pr + 99)).astype(np.float32) * SCALE
    return np.ascontiguousarray(np.concatenate([a, b], axis=1))  # [128, 256]


def _build_nc():
    nc = bacc.Bacc("TRN2", target_bir_lowering=False, debug=False)
    x_d = nc.declare_dram_parameter("x", [XROWS, W], F32R, isOutput=False)
    bands_d = nc.declare_dram_parameter("bands", [128, 256], F32R, isOutput=False)
    o_d = nc.declare_dram_parameter("out", [PER, OH, OW], F32, isOutput=True)

    with tile.TileContext(nc) as tc:
        with (
            tc.tile_pool(name="singles", bufs=1) as singles,
            tc.tile_pool(name="ximg", bufs=3) as xpool,
            tc.tile_pool(name="tbuf", bufs=4) as tpool,
            tc.tile_pool(name="scan", bufs=4) as spool,
            tc.tile_pool(name="psum", bufs=6, space="PSUM") as ppool,
        ):
            bt = singles.tile([128, 256], F32R)
            tb4 = singles.tile([128, 1224], F32)
            nc.gpsimd.memset(tb4[0:4, 0:100], 0.0)
            nc.gpsimd.dma_start(out=bt, in_=bands_d[:, :])
            ba = bt[:, 0:128]
            bb = bt[:, 128:256]

            for b in range(PER):
                # ---- one DMA per image: 9 h-chunks of 128 rows ----
                xm = xpool.tile([128, 9, XW], F32R, tag="ximg")
                for (c0, c1) in ((0, 2), (2, 5), (5, 9)):
                    nc.gpsimd.dma_start(
                        out=xm[:, c0:c1, 0:W],
                        in_=x_d[b * H + 128 * c0 : b * H + 128 * c1, :].rearrange(
                            "(t p) w -> p t w", p=128
                        ),
                    )
                # last output row (ho=1024): single matmul per chunk, then an
                # SBUF->SBUF DMA gathers the row into partition b of tb4 so a
                # single scan at the end covers all four images.
                lr = []
                for (w0, n) in WCHUNKS:
                    ps = ppool.tile([128, 512], F32, tag="psum")
                    nc.tensor.matmul(
                        ps[0:1, 0:n],
                        lhsT=ba[0:100, 0:1],
                        rhs=xm[0:100, 8, w0 : w0 + n],
                        start=True,
                        stop=True,
                    )
                    lr.append(ps)
                tbr = tpool.tile([128, 1224], F32, tag="tbuf")
                nc.scalar.copy(out=tbr[0:1, 100:612], in_=lr[0][0:1, 0:512])
                nc.scalar.copy(out=tbr[0:1, 612:1124], in_=lr[1][0:1, 0:512])
                nc.scalar.copy(out=tbr[0:1, 1124:1224], in_=lr[2][0:1, 0:100])
                nc.gpsimd.dma_start(out=tb4[b : b + 1, 100:1224], in_=tbr[0:1, 100:1224])

                for i in range(8):  # ho-tiles: 8 x 128 rows
                    M = 128
                    KA = 128

                    psums = []
                    for (w0, n) in WCHUNKS:
                        ps = ppool.tile([128, 512], F32, tag="psum")
                        nc.tensor.matmul(
                            ps[0:M, 0:n],
                            lhsT=ba[0:KA, 0:M],
                            rhs=xm[0:KA, i, w0 : w0 + n],
                            start=True,
                            stop=False,
                        )
                        KB = 128 if i < 7 else 100
                        nc.tensor.matmul(
                            ps[0:M, 0:n],
                            lhsT=bb[0:KB, 0:M],
                            rhs=xm[0:KB, i + 1, w0 : w0 + n],
                            start=False,
                            stop=True,
                        )
                        psums.append(ps)

                    # ---- assemble t row [zeros(100) | t(1124)] in SBUF ----
                    tb = tpool.tile([128, 1224], F32, tag="tbuf")
                    nc.gpsimd.memset(tb[0:M, 0:100], 0.0)
                    nc.scalar.copy(out=tb[0:M, 100:612], in_=psums[0][0:M, 0:512])
                    nc.scalar.copy(out=tb[0:M, 612:1124], in_=psums[1][0:M, 0:512])
                    nc.scalar.copy(out=tb[0:M, 1124:1224], in_=psums[2][0:M, 0:100])

                    # ---- horizontal 100-wide running box sum (one DVE op) ----
                    so = spool.tile([128, 1124], F32, tag="scan")
                    nc.vector.tensor_tensor_scan(
                        out=so[0:M, :],
                        data0=tb[0:M, 100:1224],
                        data1=tb[0:M, 0:1124],
                        initial=0.0,
                        op0=mybir.AluOpType.add,
                        op1=mybir.AluOpType.subtract,
                    )

                    nc.sync.dma_start(
                        out=o_d[b, i * 128 : i * 128 + M, :],
                        in_=so[0:M, 99:1124],
                    )
            # ---- combined last-row scan for all four images ----
            so = spool.tile([128, 1124], F32, tag="scan")
            nc.vector.tensor_tensor_scan(
                out=so[0:4, :],
                data0=tb4[0:4, 100:1224],
                data1=tb4[0:4, 0:1124],
                initial=0.0,
                op0=mybir.AluOpType.add,
                op1=mybir.AluOpType.subtract,
            )
            for b in range(PER):
                nc.sync.dma_start(
                    out=o_d[b, 1024:1025, :],
                    in_=so[b : b + 1, 99:1124],
                )
    nc.finalize()
    return nc


_CACHE = {}


def _get_nc():
    if "nc" not in _CACHE:
        _CACHE["nc"] = _build_nc()
    return _CACHE["nc"]


def _run(x4, trace=False):
    """x4: [32, 1124, 1124] float32. Returns [32, 1025, 1025] float32."""
    bands = _bands()
    in_maps = []
    for c in range(NCORES):
        xp = np.zeros((XROWS, W), dtype=np.float32)
        xp[: PER * H] = x4[PER * c : PER * (c + 1)].reshape(PER * H, W)
        in_maps.append({"x": xp, "bands": bands})
    r = run_bass_kernel_spmd(
        _get_nc(), in_maps, list(range(NCORES)), trace=trace
    )
    out = np.concatenate([r.results[c]["out"] for c in range(NCORES)], axis=0)
    return out, r


def kernel(x):
    x = np.asarray(x, dtype=np.float32).reshape(B, H, W)
    out, _ = _run(x)
    return out.reshape(B, 1, OH, OW)
